# revision 6
# baseline (speedup 1.0000x reference)
"""GQA attention block (RoPE + causal attention + out-proj) on 8 TRN2 cores.

Problem: nn_AdvancedAttn (B=2, S=2048, DIM=2048, H=16 q-heads, KVH=4 kv-heads,
DH=128), fp32 in/out.

Sharding: core (b, g) for b in {0,1}, g in {0..3} handles batch b and kv-head
group g (4 query heads + 1 kv head).  Wq/Wk/Wv are split along the head dim,
Wo along its input dim; the 4 partial Wo outputs per batch are summed on host
(the all-reduce of tensor parallelism).

Device kernel (per core), all matmuls in fp32r (full PE speed at N>=256,
~tf32 precision):
  - projections contract over DIM with x^T (host-pretransposed) streamed in
    128-row chunks; Q^T/K^T produced in [dh, s] layout with RoPE fused into
    the PSUM eviction; V produced as V^T then PE-transposed to [s, dh] chunks.
  - scores computed transposed (S^T[sk, sq] = K^T.T @ Q^T) so softmax(exp)
    tiles feed the A@V matmul directly; no max-subtraction (scores are O(10),
    exp is safe in fp32, and masked entries underflow to exactly 0).
  - row-sums via a ones-vector matmul accumulated alongside A@V; reciprocal
    broadcast across partitions with a rank-1 PE outer product.
  - mask handled data-dependently: host classifies each [128 sk, 512 sq]
    block of mask^T as all-zero (no-op), all -inf (block skipped entirely:
    no scores/exp/AV work) or mixed (DVE add of the actual values).  This
    makes the kernel causal-aware without assuming causality.
"""
import json
import math

import numpy as np

import concourse.bass as bass
import concourse.mybir as mybir
import concourse.tile as tile
from concourse.bass_utils import run_bass_kernel_spmd

# ---------------------------------------------------------------- constants
B = 2
S = 2048
DIM = 2048
H = 16
KVH = 4
DH = 128
HPC = 4           # query heads per core
NCORE = 8
THETA = 10000.0
P = 128
ST = 512          # s-tile width (sequence) for projections / attention rhs
NCH = DIM // P    # 16 contraction chunks
NST = S // ST     # 4 s-tiles
SCALE = 1.0 / math.sqrt(DH)
F32 = mybir.dt.float32
F32R = mybir.dt.float32r
BF16 = mybir.dt.bfloat16
NEG_THRESH = -1e30
# mixed mask blocks stay SBUF-resident up to this count; past it they are
# re-streamed per use (correct but slower fallback).
MAX_RESIDENT_MIXED = 24

# ------------------------------------------------- walrus multi-wait fixup
# This toolchain's walrus supports fewer sync-waits per instruction than
# Tile emits (observed: Matmult chokes at 2, Drain at 3).  Splitting excess
# waits onto NoOps on the same engine queue immediately before the
# instruction is semantically identical (the engine stalls at the NoOp).
_SKIP_OPCODES = {"CollectiveCompute"}


def _split_waits_in_bir(d: dict) -> None:
    for fn in d.get("functions", []):
        for blk in fn.get("blocks", []):
            out = []
            for inst in blk.get("instructions", []):
                si = inst.get("sync_info")
                waits = (si or {}).get("on_wait") or []
                if len(waits) > 1 and inst.get("opcode", "") not in _SKIP_OPCODES:
                    for k, w in enumerate(waits[1:]):
                        out.append({
                            "debug": inst.get("debug", 0),
                            "engine": inst["engine"],
                            "ins": [],
                            "name": f"{inst['name']}-wsplit{k}",
                            "opcode": "NoOp",
                            "outs": [],
                            "sync_info": {"on_update": [], "on_wait": [w]},
                        })
                    si["on_wait"] = waits[:1]
                out.append(inst)
            blk["instructions"] = out


_waitfix_installed = False


def _install_waitfix():
    global _waitfix_installed
    if _waitfix_installed:
        return
    orig = bass.Bass.to_json_bytes

    def to_json_bytes_split(self):
        d = json.loads(orig(self))
        _split_waits_in_bir(d)
        return json.dumps(d).encode()

    bass.Bass.to_json_bytes = to_json_bytes_split
    _waitfix_installed = True


# ------------------------------------------------------------ program build
def build_program(block_kind, mask_exact_binary=True):
    """block_kind[(c, t)] in {'allow', 'skip', int mixed-block-index} for
    sk-chunk c (16 of 128) x sq-tile t (4 of 512) of the transposed mask.
    mask_exact_binary: every mixed value is 0 or <= -1e30 (bf16-exact)."""
    _install_waitfix()
    from contextlib import ExitStack
    n_mixed = sum(1 for v in block_kind.values() if isinstance(v, int))
    resident = n_mixed <= MAX_RESIDENT_MIXED
    mdt = BF16 if mask_exact_binary else F32
    chunks_of = {t: [c for c in range(NCH) if block_kind[(c, t)] != "skip"]
                 for t in range(NST)}

    nc = bass.Bass("TRN2", target_bir_lowering=False, debug=False)
    xT = nc.declare_dram_parameter("xT", [DIM, S], F32R, isOutput=False)
    wq = nc.declare_dram_parameter("wq", [DIM, HPC * DH], F32R, isOutput=False)
    wk = nc.declare_dram_parameter("wk", [DIM, DH], F32R, isOutput=False)
    wv = nc.declare_dram_parameter("wv", [DIM, DH], F32R, isOutput=False)
    wo = nc.declare_dram_parameter("wo", [HPC * DH, DIM], F32R, isOutput=False)
    cosT = nc.declare_dram_parameter("cosT", [DH, S], F32, isOutput=False)
    sinTs = nc.declare_dram_parameter("sinTs", [DH, S], F32, isOutput=False)
    onescol = nc.declare_dram_parameter("onescol", [P, 1], F32R, isOutput=False)
    onesrow = nc.declare_dram_parameter("onesrow", [1, P], F32R, isOutput=False)
    ident = nc.declare_dram_parameter("ident", [P, P], F32R, isOutput=False)
    if n_mixed:
        mmask = nc.declare_dram_parameter(
            "mmask", [n_mixed, P, ST], mdt, isOutput=False)
    y = nc.declare_dram_parameter("y", [S, DIM], F32, isOutput=True)

    allp = mybir.AluOpType
    AF = mybir.ActivationFunctionType

    with tile.TileContext(nc) as tc, ExitStack() as ctx:
        # ---- persistent pool -----------------------------------------
        keep = ctx.enter_context(tc.tile_pool(name="keep", bufs=1))
        ones_c = keep.tile([P, 1], F32R)
        nc.sync.dma_start(out=ones_c[:], in_=onescol[:])
        ones_r = keep.tile([1, P], F32R)
        nc.sync.dma_start(out=ones_r[:], in_=onesrow[:])
        qT = keep.tile([P, HPC, S], F32R)    # Q^T per head, RoPEd
        kT = keep.tile([P, S], F32R)         # K^T, RoPEd
        vsb = keep.tile([P, NCH, DH], F32R)  # V in [s, dh] chunks
        oT = keep.tile([P, HPC, S], F32R)    # attention out^T per head
        if n_mixed and resident:
            mm_sb = keep.tile([P, n_mixed, ST], mdt)
            nc.sync.dma_start(
                out=mm_sb[:], in_=mmask[:].rearrange("n p m -> p n m"))

        # ---- projection phase ----------------------------------------
        with ExitStack() as pctx:
            wpool = pctx.enter_context(tc.tile_pool(name="wpool", bufs=1))
            wq_sb = wpool.tile([P, NCH, HPC * DH], F32R)
            wk_sb = wpool.tile([P, NCH, DH], F32R)
            wv_sb = wpool.tile([P, NCH, DH], F32R)
            wq3 = wq[:].rearrange("(c p) m -> p c m", p=P)
            wk3 = wk[:].rearrange("(c p) m -> p c m", p=P)
            wv3 = wv[:].rearrange("(c p) m -> p c m", p=P)
            for c in range(NCH):
                nc.sync.dma_start(out=wq_sb[:, c, :], in_=wq3[:, c, :])
                nc.sync.dma_start(out=wk_sb[:, c, :], in_=wk3[:, c, :])
                nc.sync.dma_start(out=wv_sb[:, c, :], in_=wv3[:, c, :])
            id_sb = wpool.tile([P, P], F32R)
            nc.sync.dma_start(out=id_sb[:], in_=ident[:])
            vT = wpool.tile([P, S], F32R)    # V^T staging

            xpool = pctx.enter_context(tc.tile_pool(name="xpool", bufs=4))
            rpool = pctx.enter_context(tc.tile_pool(name="rope", bufs=4))
            tabp = pctx.enter_context(tc.tile_pool(name="tabs", bufs=2))

            def rope_evict(dst, ps, cos_t, sin_t):
                # dst = ps * cos + rotate_half(ps) * sin  (sign baked into
                # sin_t: rows 0..63 hold -sin, 64..127 hold +sin)
                tmp = rpool.tile([P, ST], F32, tag="ropetmp", name="ropetmp")
                nc.vector.tensor_tensor(
                    out=tmp[0:64, :], in0=ps[64:128, :],
                    in1=sin_t[0:64, :], op=allp.mult)
                nc.vector.tensor_tensor(
                    out=tmp[64:128, :], in0=ps[0:64, :],
                    in1=sin_t[64:128, :], op=allp.mult)
                with nc.allow_low_precision(reason="f32r 32-bit storage"):
                    nc.vector.tensor_tensor(
                        out=dst, in0=ps[:], in1=cos_t[:], op=allp.mult)
                    nc.vector.tensor_tensor(
                        out=dst, in0=dst, in1=tmp[:], op=allp.add)

            with tc.tile_pool(name="pp", bufs=8, space="PSUM") as pp:
                for st in range(NST):
                    cols = bass.ts(st, ST)
                    cos_t = tabp.tile([P, ST], F32, tag="cos", name="cos_t")
                    nc.sync.dma_start(out=cos_t[:], in_=cosT[:, cols])
                    sin_t = tabp.tile([P, ST], F32, tag="sin", name="sin_t")
                    nc.sync.dma_start(out=sin_t[:], in_=sinTs[:, cols])
                    ps_list = [pp.tile([P, ST], F32, tag="proj",
                                       name=f"proj{j}")
                               for j in range(6)]
                    for c in range(NCH):
                        xt = xpool.tile([P, ST], F32R, tag="xt", name="xt")
                        nc.gpsimd.dma_start(
                            out=xt[:], in_=xT[c * P:(c + 1) * P, cols])
                        for hh in range(HPC):
                            nc.tensor.matmul(
                                ps_list[hh][:],
                                wq_sb[:, c, hh * DH:(hh + 1) * DH],
                                xt[:], start=(c == 0), stop=(c == NCH - 1))
                        nc.tensor.matmul(
                            ps_list[4][:], wk_sb[:, c, :], xt[:],
                            start=(c == 0), stop=(c == NCH - 1))
                        nc.tensor.matmul(
                            ps_list[5][:], wv_sb[:, c, :], xt[:],
                            start=(c == 0), stop=(c == NCH - 1))
                    for hh in range(HPC):
                        rope_evict(qT[:, hh, cols], ps_list[hh][:],
                                   cos_t, sin_t)
                    rope_evict(kT[:, cols], ps_list[4][:], cos_t, sin_t)
                    nc.scalar.activation(
                        out=vT[:, cols], in_=ps_list[5][:], func=AF.Copy)

            # V^T -> V chunks via PE transpose
            with tc.tile_pool(name="pt", bufs=2, space="PSUM") as pt:
                for c in range(NCH):
                    tp = pt.tile([P, P], F32R, tag="tp", name="tp")
                    nc.tensor.transpose(
                        tp[:], vT[:, c * P:(c + 1) * P], id_sb[:])
                    nc.scalar.activation(
                        out=vsb[:, c, :], in_=tp[:], func=AF.Copy)

        # ---- attention phase -----------------------------------------
        with ExitStack() as actx:
            apool = actx.enter_context(tc.tile_pool(name="apool", bufs=1))
            wo_sb = apool.tile([P, HPC, DIM], F32R)
            nc.sync.dma_start(
                out=wo_sb[:], in_=wo[:].rearrange("(h p) n -> p h n", p=P))

            a2 = actx.enter_context(ExitStack())
            epool = a2.enter_context(tc.tile_pool(name="epool", bufs=3))
            tpool = a2.enter_context(tc.tile_pool(name="tpool", bufs=4))
            rsp = a2.enter_context(tc.tile_pool(name="rsp", bufs=2))
            if n_mixed and not resident:
                mstr = a2.enter_context(tc.tile_pool(name="mstr", bufs=4))
            pc = a2.enter_context(tc.tile_pool(name="pc", bufs=2, space="PSUM"))
            po = a2.enter_context(tc.tile_pool(name="po", bufs=2, space="PSUM"))
            pr = a2.enter_context(tc.tile_pool(name="pr", bufs=1, space="PSUM"))
            pb = a2.enter_context(tc.tile_pool(name="pb", bufs=1, space="PSUM"))

            for hh in range(HPC):
                for t in range(NST):
                    cols = bass.ts(t, ST)
                    chunks = chunks_of[t]
                    pairs = [chunks[i:i + 2] for i in range(0, len(chunks), 2)]
                    ps_o = po.tile([P, ST], F32, tag="o", name="ps_o")
                    ps_r = pr.tile([1, ST], F32, tag="r", name="ps_r")
                    ci = 0
                    for pair in pairs:
                        w = len(pair) * ST
                        ps_c = pc.tile([P, 2 * ST], F32, tag="c", name="ps_c")
                        for j, c in enumerate(pair):
                            nc.tensor.matmul(
                                ps_c[:, j * ST:(j + 1) * ST],
                                kT[:, c * P:(c + 1) * P],
                                qT[:, hh, cols], start=True, stop=True)
                            kind = block_kind[(c, t)]
                            if isinstance(kind, int):
                                if resident:
                                    msl = mm_sb[:, kind, :]
                                else:
                                    mtile = mstr.tile([P, ST], mdt, tag="ms",
                                                      name="mtile")
                                    nc.sync.dma_start(
                                        out=mtile[:], in_=mmask[kind])
                                    msl = mtile[:]
                                nc.vector.tensor_tensor(
                                    out=ps_c[:, j * ST:(j + 1) * ST],
                                    in0=ps_c[:, j * ST:(j + 1) * ST],
                                    in1=msl, op=allp.add)
                        et = epool.tile([P, 2 * ST], F32R, tag="e", name="et")
                        nc.scalar.activation(
                            out=et[:, :w], in_=ps_c[:, :w],
                            func=AF.Exp, scale=SCALE)
                        for j, c in enumerate(pair):
                            first = (ci == 0)
                            last = (ci == len(chunks) - 1)
                            esl = et[:, j * ST:(j + 1) * ST]
                            nc.tensor.matmul(
                                ps_o[:], vsb[:, c, :], esl,
                                start=first, stop=last)
                            nc.tensor.matmul(
                                ps_r[:], ones_c[:], esl,
                                start=first, stop=last)
                            ci += 1
                    rs = rsp.tile([1, ST], F32R, tag="rs", name="rs")
                    with nc.allow_low_precision(reason="f32r storage"):
                        nc.vector.reciprocal(out=rs[:], in_=ps_r[:])
                    ps_b = pb.tile([P, ST], F32, tag="b", name="ps_b")
                    nc.tensor.matmul(
                        ps_b[:], ones_r[:], rs[:], start=True, stop=True)
                    bc = tpool.tile([P, ST], F32, tag="bc", name="bc")
                    nc.scalar.activation(out=bc[:], in_=ps_b[:], func=AF.Copy)
                    with nc.allow_low_precision(reason="f32r storage"):
                        nc.vector.tensor_tensor(
                            out=oT[:, hh, cols], in0=ps_o[:], in1=bc[:],
                            op=allp.mult)

            # ---- output projection -----------------------------------
            a2.close()
            ypool = actx.enter_context(tc.tile_pool(name="ypool", bufs=4))
            with tc.tile_pool(name="py", bufs=4, space="PSUM") as py:
                for tq in range(S // P):
                    rows = bass.ts(tq, P)
                    for n in range(DIM // ST):
                        ps_y = py.tile([P, ST], F32, tag="y", name="ps_y")
                        for hh in range(HPC):
                            nc.tensor.matmul(
                                ps_y[:], oT[:, hh, rows],
                                wo_sb[:, hh, n * ST:(n + 1) * ST],
                                start=(hh == 0), stop=(hh == HPC - 1))
                        ysb = ypool.tile([P, ST], F32, tag="ys", name="ysb")
                        nc.vector.tensor_copy(ysb[:], ps_y[:])
                        nc.gpsimd.dma_start(
                            out=y[tq * P:(tq + 1) * P, n * ST:(n + 1) * ST],
                            in_=ysb[:])
    return nc


# ------------------------------------------------------------- host driver
def _classify_mask(mask2d):
    """Classify [128, 512] blocks of mask^T.

    Returns (block_kind, mixed_vals, exact_binary): block_kind[(c, t)] is
    'allow' | 'skip' | index into mixed_vals; mixed_vals is
    [n_mixed, 128, 512] of mask^T values pre-scaled by sqrt(DH) (the exp
    applies scale 1/sqrt(DH) to scores + mask alike), clipped to stay
    finite; exact_binary is True when every mixed value is 0 or <= -1e30
    (bf16 then keeps the semantics exactly)."""
    mT = mask2d.T  # [sk, sq]
    block_kind = {}
    mixed = []
    exact_binary = True
    for c in range(NCH):
        for t in range(NST):
            blk = mT[c * P:(c + 1) * P, t * ST:(t + 1) * ST]
            if not blk.any():
                block_kind[(c, t)] = "allow"
            elif (blk <= NEG_THRESH).all():
                block_kind[(c, t)] = "skip"
            else:
                block_kind[(c, t)] = len(mixed)
                if not ((blk == 0) | (blk <= NEG_THRESH)).all():
                    exact_binary = False
                scaled = np.clip(
                    blk.astype(np.float64) * math.sqrt(DH), -3e38, 3e38
                ).astype(np.float32)
                mixed.append(scaled)
    mv = np.stack(mixed) if mixed else None
    return block_kind, mv, exact_binary


def _rope_tables(position_ids):
    pos = position_ids.reshape(-1).astype(np.float64)  # [S]
    inv = 1.0 / (THETA ** (np.arange(0, DH, 2, dtype=np.float64) / DH))
    fr = pos[None, :] * inv[:, None]          # [64, S]
    cosT = np.empty((DH, S), np.float32)
    sinTs = np.empty((DH, S), np.float32)
    cosT[0:64] = np.cos(fr)
    cosT[64:128] = np.cos(fr)
    sinTs[0:64] = -np.sin(fr)
    sinTs[64:128] = np.sin(fr)
    return cosT, sinTs


def _prep_inputs(x, mask, position_ids, Wq, Wk, Wv, Wo):
    cosT, sinTs = _rope_tables(np.asarray(position_ids))
    block_kind, mixed_vals, exact_binary = _classify_mask(
        np.asarray(mask)[0, 0])
    if mixed_vals is not None and exact_binary:
        import ml_dtypes
        # bf16 keeps the 0 / -huge semantics exactly (huge negatives round
        # to -inf or stay huge; exp underflows to 0 either way)
        mixed_vals = mixed_vals.astype(ml_dtypes.bfloat16)
    onescol = np.ones((P, 1), np.float32)
    onesrow = np.ones((1, P), np.float32)
    ident = np.eye(P, dtype=np.float32)
    in_maps = []
    for core in range(NCORE):
        b, g = divmod(core, KVH)
        m = {
            "xT": np.ascontiguousarray(np.asarray(x)[b].T),
            "wq": np.ascontiguousarray(Wq[:, g * HPC * DH:(g + 1) * HPC * DH]),
            "wk": np.ascontiguousarray(Wk[:, g * DH:(g + 1) * DH]),
            "wv": np.ascontiguousarray(Wv[:, g * DH:(g + 1) * DH]),
            "wo": np.ascontiguousarray(Wo[g * HPC * DH:(g + 1) * HPC * DH, :]),
            "cosT": cosT, "sinTs": sinTs,
            "onescol": onescol, "onesrow": onesrow, "ident": ident,
        }
        if mixed_vals is not None:
            m["mmask"] = mixed_vals
        in_maps.append(m)
    return (block_kind, exact_binary), in_maps


def kernel(x, mask, position_ids, Wq, Wk, Wv, Wo):
    (block_kind, exact_binary), in_maps = _prep_inputs(
        x, mask, position_ids, Wq, Wk, Wv, Wo)
    nc = build_program(block_kind, mask_exact_binary=exact_binary)
    res = run_bass_kernel_spmd(nc, in_maps, core_ids=list(range(NCORE)))
    out = np.zeros((B, S, DIM), np.float32)
    for core in range(NCORE):
        b = core // KVH
        out[b] += res.results[core]["y"]
    return out
